# revision 9
# baseline (speedup 1.0000x reference)
"""Trainium2 Bass kernel for nn_Blast: out = x @ (W0 + 1 bias^T) + bias
where W0 block (i_in, i_out) = Vt[i] @ diag(S[o,i]) @ U[o].

v8: y-factorization. Per core (256 tokens):
  phase A: yT[(i,r), t] = blockdiag(Vt)^T @ xT   (32 matmuls, M=32 strips,
           col 16 of each strip = ones -> per-block rowsum rides along)
  z-step:  zT[(o,r), t] = smat^T @ ysb           (16 matmuls, smat is the
           host-built S scatter incl rowsum->row16 and +1 via const row 31)
  phase B: out[t, 256o+q] = sum_k zsb[k, t] usb[k, 256o+q]
           (usb = block-diag U built on gpsimd; row 16 = bias via DMA ->
           the (rowsum+1)*bias term rides every o-block matmul)

Row pitch is 32 everywhere (engine partition accesses must start at a
multiple of 32): y tile tau strip j holds i = 4*tau+j (rows 16 ranks,
rowsum_i at 16, zeros above); zsb/usb bank b holds o = 4b..4b+3 at rows
32*o_loc (+ rowsum+1 / bias at row 16). ysb row 31 of tile 0 = 1.0 via
a tiny DMA (the +1 source; engine memset can't start at partition 31).

DMA: x in 5 batches on sync (tiny canary first: receipts fire ~2.3us
after last byte); weights (vt/smat/uc/biasrow) on scalar. Out in 8
[128,1024] pieces on sync as phase-B PSUM->SBUF copies land (DVE/ACT
alternate; they are the phase-B pacing at ~96G elem/s each).
"""

import numpy as np

IN_DIM = 4096
OUT_DIM = 4096
BLOCK = 256
RANK = 16
B_IN = 16
B_OUT = 16
N_CORES = 8
TOK = 2048
TPC = TOK // N_CORES          # 256 tokens per core
NCHUNK = IN_DIM // 128        # 32 K-chunks
NTAU = 4                      # y psum tiles (4 i-blocks each)
NBANK = 4                     # zsb banks: 4 o-blocks each, pitch 32
BROW = 16                     # bias/rowsum row in zsb/usb (per bank)
XBATCH = [1, 7, 8, 8, 8]      # x chunks per DMA (canary first)
NWARM = 16                    # K=128 N=256 warm matmuls (HAM un-throttle)

_CACHE = {}

# test.py toggles; harness never touches these
TRACE = False
TRACE_DIR = None
LAST_RESULTS = None


def _bank(o):
    return o // 4, o % 4


def build_program():
    import concourse.mybir as mybir
    from concourse import bacc
    from concourse.tile import TileContext

    bf16 = mybir.dt.bfloat16
    f32 = mybir.dt.float32

    nc = bacc.Bacc(trn_type="TRN2")
    # xt pre-interleaved on host: xt[p, k*TPC+t] = x^T[128k+p, t]
    xt_d = nc.dram_tensor("xt", (128, NCHUNK * TPC), bf16, kind="ExternalInput")
    vt_d = nc.dram_tensor("vt", (128, NCHUNK * 32), bf16, kind="ExternalInput")
    smat_d = nc.dram_tensor(
        "smat", (128, NTAU * NBANK * 128), bf16, kind="ExternalInput"
    )
    uc_d = nc.dram_tensor("uc", (128, NBANK * BLOCK), bf16, kind="ExternalInput")
    br_d = nc.dram_tensor("br", (1, OUT_DIM), bf16, kind="ExternalInput")
    ones_d = nc.dram_tensor("ones", (1, TPC), bf16, kind="ExternalInput")
    out_d = nc.dram_tensor("out", (TPC, OUT_DIM), bf16, kind="ExternalOutput")

    with TileContext(nc) as tc:
        from contextlib import ExitStack

        with ExitStack() as ctx:
            consts = ctx.enter_context(tc.tile_pool(name="consts", bufs=1))
            xpool = ctx.enter_context(tc.tile_pool(name="xpool", bufs=1))
            outsb = ctx.enter_context(tc.tile_pool(name="outsb", bufs=1))
            ps_y = ctx.enter_context(tc.tile_pool(name="ps_y", bufs=1, space="PSUM"))
            ps_z = ctx.enter_context(tc.tile_pool(name="ps_z", bufs=1, space="PSUM"))

            # ---- SBUF tiles ----
            wsrc = consts.tile([128, TPC], bf16, name="wsrc", tag="wsrc")
            nc.vector.memset(wsrc[:], 0.0)

            vt_sb = consts.tile([128, NCHUNK * 32], bf16, name="vt_sb", tag="vt_sb")
            smat_sb = consts.tile(
                [128, NTAU * NBANK * 128], bf16, name="smat_sb", tag="smat_sb"
            )
            uc_sb = consts.tile([128, NBANK * BLOCK], bf16, name="uc_sb", tag="uc_sb")
            usb = consts.tile([128, OUT_DIM], bf16, name="usb", tag="usb")
            ysb = consts.tile([128, NTAU * TPC], bf16, name="ysb", tag="ysb")
            zsb = consts.tile([128, NBANK * TPC], bf16, name="zsb", tag="zsb")

            nc.gpsimd.memset(usb[:], 0.0)

            # ---- input DMAs ----
            # scalar ring: weights (bias row last: overwrites usb row 16
            # after the gpsimd block-diag copies)
            nc.scalar.dma_start(out=vt_sb[:], in_=vt_d[:])
            nc.scalar.dma_start(out=smat_sb[:], in_=smat_d[:])
            nc.scalar.dma_start(out=uc_sb[:], in_=uc_d[:])

            # sync ring: x batches (canary first)
            xbatches = []
            xoff = []
            k0 = 0
            for bi, nk in enumerate(XBATCH):
                xb = xpool.tile([128, nk * TPC], bf16, name=f"xb{k0}", tag=f"xb{k0}")
                nc.sync.dma_start(out=xb[:], in_=xt_d[:, k0 * TPC : (k0 + nk) * TPC])
                xbatches.append(xb)
                xoff.append(k0)
                k0 += nk

            def xchunk(k):
                for xb, o in zip(xbatches, xoff):
                    nk = xb.shape[1] // TPC
                    if o <= k < o + nk:
                        return xb[:, (k - o) * TPC : (k - o + 1) * TPC]
                raise AssertionError

            # ---- usb build on gpsimd: block-diag U from pre-placed uc ----
            # copy rows 32*o_loc..+32 (gap rows are zero in uc; bias DMA
            # fills row 16 afterwards)
            for o in range(B_OUT):
                b, o_loc = _bank(o)
                r0 = 32 * o_loc
                nc.gpsimd.tensor_copy(
                    usb[r0 : r0 + 32, BLOCK * o : BLOCK * (o + 1)],
                    uc_sb[r0 : r0 + 32, BLOCK * b : BLOCK * (b + 1)],
                )
            # bias row AFTER the o_loc=0 copies (they cover row 16)
            nc.scalar.dma_start(out=usb[BROW : BROW + 1, :], in_=br_d[:])

            # PSUM: 2 y banks + 2 z banks + 4 out banks = 8 exactly.
            # Warm matmuls share the z bank region (z starts after warms).
            ypair = [
                ps_y.tile([128, 2 * TPC], f32, name=f"yp{t}", tag=f"yp{t}")
                for t in range(2)
            ]
            zpair = [
                ps_z.tile([128, 2 * TPC], f32, name=f"zp{t}", tag=f"zp{t}")
                for t in range(2)
            ]

            def ytile(tau):
                return ypair[tau // 2][:, TPC * (tau % 2) : TPC * (tau % 2 + 1)]

            def ztile(b):
                return zpair[b // 2][:, TPC * (b % 2) : TPC * (b % 2 + 1)]

            # ---- PE warmup (HAM un-throttle; covers x canary receipt) ----
            warm = zpair[1][:, 0:TPC]
            for _ in range(NWARM):
                nc.tensor.matmul(
                    warm,
                    lhsT=wsrc[:, 0:128],
                    rhs=wsrc[:],
                    start=True,
                    stop=True,
                    tile_position=(0, 0),
                )

            # ---- phase A: yT strips ----
            for k in range(NCHUNK):
                i = k // 2
                tau, j = i // 4, i % 4
                nc.tensor.matmul(
                    ytile(tau)[32 * j : 32 * j + 32, :],
                    lhsT=vt_sb[:, 32 * k : 32 * k + 32],
                    rhs=xchunk(k),
                    start=(k % 2 == 0),
                    stop=(k % 2 == 1),
                    tile_position=(0, 32 * j),
                )

            # ---- y -> SBUF (bf16), alternate DVE/ACT; +1 const row ----
            for t in range(NTAU):
                dst = ysb[:, TPC * t : TPC * (t + 1)]
                if t % 2 == 0:
                    nc.vector.tensor_copy(dst, ytile(t))
                else:
                    nc.scalar.copy(dst, ytile(t))
            # const-1 row for the '+1' bias term (sync ring, after y0 copy)
            nc.sync.dma_start(out=ysb[31:32, 0:TPC], in_=ones_d[:])

            # ---- z-step: zT = smat^T @ ysb ----
            # bank-major: a start=True clears the whole PSUM bank's
            # has_written, so groups sharing a bank must not interleave
            for b in range(NBANK):
                for tau in range(NTAU):
                    nc.tensor.matmul(
                        ztile(b),
                        lhsT=smat_sb[
                            :, 128 * (NBANK * tau + b) : 128 * (NBANK * tau + b + 1)
                        ],
                        rhs=ysb[:, TPC * tau : TPC * (tau + 1)],
                        start=(tau == 0),
                        stop=(tau == NTAU - 1),
                        tile_position=(0, 0),
                    )

            # ---- z -> SBUF ----
            for b in range(NBANK):
                dst = zsb[:, TPC * b : TPC * (b + 1)]
                if b % 2 == 0:
                    nc.vector.tensor_copy(dst, ztile(b))
                else:
                    nc.scalar.copy(dst, ztile(b))

            # ---- phase B: out[t, oq] per o-block; copies pace; flush out ----
            ps_out = ctx.enter_context(
                tc.tile_pool(name="ps_out", bufs=4, space="PSUM")
            )
            for half in range(2):
                osb_t = outsb.tile(
                    [128, OUT_DIM], bf16, name=f"osb{half}", tag=f"osb{half}"
                )
                row = slice(half * 128, (half + 1) * 128)
                for m in range(8):
                    po = ps_out.tile([128, 512], f32, name="po", tag="po")
                    for o in (2 * m, 2 * m + 1):
                        b, _ = _bank(o)
                        nc.tensor.matmul(
                            po[:, (o % 2) * BLOCK : (o % 2 + 1) * BLOCK],
                            lhsT=zsb[
                                :, TPC * b + 128 * half : TPC * b + 128 * (half + 1)
                            ],
                            rhs=usb[:, BLOCK * o : BLOCK * (o + 1)],
                            start=True,
                            stop=True,
                            tile_position=(0, 0),
                        )
                    dst = osb_t[:, 512 * m : 512 * (m + 1)]
                    if m % 2 == 0:
                        nc.vector.tensor_copy(dst, po[:])
                    else:
                        nc.scalar.copy(dst, po[:])
                    if m % 2 == 1:
                        c0 = 512 * (m - 1)
                        nc.sync.dma_start(
                            out=out_d[row, c0 : c0 + 1024],
                            in_=osb_t[:, c0 : c0 + 1024],
                        )

    nc.compile()
    return nc


def prep_inputs(x, S, U, Vt, bias):
    """Host-side layout prep (bf16). Returns per-core input maps."""
    import ml_dtypes

    bf = ml_dtypes.bfloat16
    x = np.asarray(x, dtype=np.float32)
    S = np.asarray(S, dtype=np.float32)
    U = np.asarray(U, dtype=np.float32)
    Vt = np.asarray(Vt, dtype=np.float32)
    bias = np.asarray(bias, dtype=np.float32)

    xt = np.ascontiguousarray(x.reshape(TOK, IN_DIM).T).astype(bf)  # (4096, 2048)

    # vt[p, 32k + c]: c<16 -> Vt[i, 128h+p, c] (k=2i+h); c==16 -> 1.0; else 0
    vt_host = np.zeros((128, NCHUNK, 32), np.float32)
    for k in range(NCHUNK):
        i, h = k // 2, k % 2
        vt_host[:, k, 0:RANK] = Vt[i, 128 * h : 128 * (h + 1), :]
        vt_host[:, k, 16] = 1.0
    vt_host = vt_host.reshape(128, NCHUNK * 32).astype(bf)

    # smat block (tau, b) at cols 128*(NBANK*tau + b):
    #   [32j + r, 32 o_loc + r] = S[o, 4 tau + j, r]
    #   [32j + 16, BROW] = 1.0 (rowsum accumulate); [31, BROW] = 1.0 on tau=0
    smat = np.zeros((128, NTAU * NBANK * 128), np.float32)
    for tau in range(NTAU):
        for b in range(NBANK):
            c0 = 128 * (NBANK * tau + b)
            for j in range(4):
                i = 4 * tau + j
                for o in range(4 * b, 4 * b + 4):
                    o_loc = o % 4
                    for r in range(RANK):
                        smat[32 * j + r, c0 + 32 * o_loc + r] = S[o, i, r]
                smat[32 * j + 16, c0 + BROW] = 1.0
            if tau == 0:
                smat[31, c0 + BROW] = 1.0
    smat = smat.astype(bf)

    # uc[32 o_loc + r, 256 b + q] = U[o, r, q] (pre-placed rows per bank)
    uc = np.zeros((128, NBANK * BLOCK), np.float32)
    for o in range(B_OUT):
        b, o_loc = _bank(o)
        uc[32 * o_loc : 32 * o_loc + RANK, BLOCK * b : BLOCK * (b + 1)] = U[o]
    uc = uc.astype(bf)

    br = bias.reshape(1, OUT_DIM).astype(bf)
    ones = np.ones((1, TPC), np.float32).astype(bf)

    in_maps = []
    for c in range(N_CORES):
        in_maps.append(
            {
                "xt": np.ascontiguousarray(
                    xt[:, c * TPC : (c + 1) * TPC]
                    .reshape(NCHUNK, 128, TPC)
                    .transpose(1, 0, 2)
                    .reshape(128, NCHUNK * TPC)
                ),
                "vt": vt_host,
                "smat": smat,
                "uc": uc,
                "br": br,
                "ones": ones,
            }
        )
    return in_maps


def kernel(x, S, U, Vt, bias):
    global LAST_RESULTS
    from concourse.bass_utils import run_bass_kernel_spmd

    if "nc" not in _CACHE:
        _CACHE["nc"] = build_program()
    nc = _CACHE["nc"]

    in_maps = prep_inputs(x, S, U, Vt, bias)
    res = run_bass_kernel_spmd(
        nc, in_maps, list(range(N_CORES)), trace=TRACE, tmpdir=TRACE_DIR
    )
    LAST_RESULTS = res
    out = np.concatenate(
        [np.asarray(res.results[c]["out"]).astype(np.float32) for c in range(N_CORES)],
        axis=0,
    )
    return out.reshape(2, TOK // 2, OUT_DIM)


# revision 14
# speedup vs baseline: 1.0867x; 1.0867x over previous
"""Trainium2 Bass kernel for nn_Blast: out = x @ (W0 + 1 bias^T) + bias
where W0 block (i_in, i_out) = Vt[i] @ diag(S[o,i]) @ U[o].

v8: y-factorization. Per core (256 tokens):
  phase A: yT[(i,r), t] = blockdiag(Vt)^T @ xT   (32 matmuls, M=32 strips,
           col 16 of each strip = ones -> per-block rowsum rides along)
  z-step:  zT[(o,r), t] = smat^T @ ysb           (16 matmuls, smat is the
           host-built S scatter incl rowsum->row16 and +1 via const row 31)
  phase B: out[t, 256o+q] = sum_k zsb[k, t] usb[k, 256o+q]
           (usb = block-diag U built on gpsimd; row 16 = bias via DMA ->
           the (rowsum+1)*bias term rides every o-block matmul)

Row pitch is 32 everywhere (engine partition accesses must start at a
multiple of 32): y tile tau strip j holds i = 4*tau+j (rows 16 ranks,
rowsum_i at 16, zeros above); zsb/usb bank b holds o = 4b..4b+3 at rows
32*o_loc (+ rowsum+1 / bias at row 16). ysb row 31 of tile 0 = 1.0 via
a tiny DMA (the +1 source; engine memset can't start at partition 31).

DMA: x in 5 batches on sync (tiny canary first: receipts fire ~2.3us
after last byte); weights (vt/smat/uc/biasrow) on scalar. Out in 8
[128,1024] pieces on sync as phase-B PSUM->SBUF copies land (DVE/ACT
alternate; they are the phase-B pacing at ~96G elem/s each).
"""

import numpy as np

IN_DIM = 4096
OUT_DIM = 4096
BLOCK = 256
RANK = 16
B_IN = 16
B_OUT = 16
N_CORES = 8
TOK = 2048
TPC = TOK // N_CORES          # 256 tokens per core
NCHUNK = IN_DIM // 128        # 32 K-chunks
NTAU = 4                      # y psum tiles (4 i-blocks each)
NBANK = 4                     # zsb banks: 4 o-blocks each, pitch 32
BROW = 16                     # bias/rowsum row in zsb/usb (per bank)
XBATCH = [1, 7, 8, 8, 8]      # x chunks per DMA (canary first)
NWARM = 26                    # K=128 N=256 warm matmuls (HAM un-throttle)

_CACHE = {}

# test.py toggles; harness never touches these
TRACE = False
TRACE_DIR = None
LAST_RESULTS = None


def _bank(o):
    return o // 4, o % 4


def build_program():
    import concourse.mybir as mybir
    from concourse import bacc
    from concourse.tile import TileContext

    bf16 = mybir.dt.bfloat16
    f32 = mybir.dt.float32

    nc = bacc.Bacc(trn_type="TRN2")
    # xt pre-interleaved on host: xt[p, k*TPC+t] = x^T[128k+p, t]
    xt_d = nc.dram_tensor("xt", (128, NCHUNK * TPC), bf16, kind="ExternalInput")
    vt_d = nc.dram_tensor("vt", (128, NCHUNK * 32), bf16, kind="ExternalInput")
    smat_d = nc.dram_tensor(
        "smat", (128, NTAU * NBANK * 128), bf16, kind="ExternalInput"
    )
    uc_d = nc.dram_tensor("uc", (128, NBANK * BLOCK), bf16, kind="ExternalInput")
    br_d = nc.dram_tensor("br", (1, OUT_DIM), bf16, kind="ExternalInput")
    ones_d = nc.dram_tensor("ones", (1, TPC), bf16, kind="ExternalInput")
    out_d = nc.dram_tensor("out", (TPC, OUT_DIM), bf16, kind="ExternalOutput")

    with TileContext(nc) as tc:
        from contextlib import ExitStack

        with ExitStack() as ctx:
            consts = ctx.enter_context(tc.tile_pool(name="consts", bufs=1))
            xpool = ctx.enter_context(tc.tile_pool(name="xpool", bufs=1))
            outsb = ctx.enter_context(tc.tile_pool(name="outsb", bufs=1))
            ps_y = ctx.enter_context(tc.tile_pool(name="ps_y", bufs=1, space="PSUM"))
            ps_z = ctx.enter_context(tc.tile_pool(name="ps_z", bufs=1, space="PSUM"))

            # ---- SBUF tiles ----
            wsrc = consts.tile([128, TPC], bf16, name="wsrc", tag="wsrc")
            nc.vector.memset(wsrc[:], 0.0)

            vt_sb = consts.tile([128, NCHUNK * 32], bf16, name="vt_sb", tag="vt_sb")
            smat_sb = consts.tile(
                [128, NTAU * NBANK * 128], bf16, name="smat_sb", tag="smat_sb"
            )
            uc_sb = consts.tile([128, NBANK * BLOCK], bf16, name="uc_sb", tag="uc_sb")
            usb = consts.tile([128, OUT_DIM], bf16, name="usb", tag="usb")
            ysb = consts.tile([128, NTAU * TPC], bf16, name="ysb", tag="ysb")
            zsb = consts.tile([128, NBANK * TPC], bf16, name="zsb", tag="zsb")

            nc.vector.memset(usb[:], 0.0)

            # ---- input DMAs ----
            # scalar ring: ones row first (no deps; y0 copy skips row 31
            # so this never waits), then weights, bias row last
            nc.scalar.dma_start(out=ysb[31:32, 0:TPC], in_=ones_d[:])
            nc.scalar.dma_start(out=vt_sb[:], in_=vt_d[:])
            nc.scalar.dma_start(out=smat_sb[:], in_=smat_d[:])
            nc.scalar.dma_start(out=uc_sb[:], in_=uc_d[:])

            # sync ring: x batches (canary first)
            xbatches = []
            xoff = []
            k0 = 0
            for bi, nk in enumerate(XBATCH):
                xb = xpool.tile([128, nk * TPC], bf16, name=f"xb{k0}", tag=f"xb{k0}")
                nc.sync.dma_start(out=xb[:], in_=xt_d[:, k0 * TPC : (k0 + nk) * TPC])
                xbatches.append(xb)
                xoff.append(k0)
                k0 += nk

            def xchunk(k):
                for xb, o in zip(xbatches, xoff):
                    nk = xb.shape[1] // TPC
                    if o <= k < o + nk:
                        return xb[:, (k - o) * TPC : (k - o + 1) * TPC]
                raise AssertionError

            # ---- usb build on DVE: block-diag U from pre-placed uc ----
            # (gpsimd is ~1us/op for small copies — unusable). Copy rows
            # 32*o_loc..+32 (gap rows are zero in uc; bias DMA fills row
            # 16 afterwards)
            for o in range(B_OUT):
                b, o_loc = _bank(o)
                r0 = 32 * o_loc
                nc.vector.tensor_copy(
                    usb[r0 : r0 + 32, BLOCK * o : BLOCK * (o + 1)],
                    uc_sb[r0 : r0 + 32, BLOCK * b : BLOCK * (b + 1)],
                )
            # bias row AFTER the o_loc=0 copies (they cover row 16)
            nc.scalar.dma_start(out=usb[BROW : BROW + 1, :], in_=br_d[:])

            # PSUM: 2 y banks + 2 z banks + 4 out banks = 8 exactly.
            # Warm matmuls share the z bank region (z starts after warms).
            ypair = [
                ps_y.tile([128, 2 * TPC], f32, name=f"yp{t}", tag=f"yp{t}")
                for t in range(2)
            ]
            zpair = [
                ps_z.tile([128, 2 * TPC], f32, name=f"zp{t}", tag=f"zp{t}")
                for t in range(2)
            ]

            def ytile(tau):
                return ypair[tau // 2][:, TPC * (tau % 2) : TPC * (tau % 2 + 1)]

            def ztile(b):
                return zpair[b // 2][:, TPC * (b % 2) : TPC * (b % 2 + 1)]

            # ---- PE warmup (HAM un-throttle; covers x canary receipt) ----
            warm = zpair[1][:, 0:TPC]
            for _ in range(NWARM):
                nc.tensor.matmul(
                    warm,
                    lhsT=wsrc[:, 0:128],
                    rhs=wsrc[:],
                    start=True,
                    stop=True,
                    tile_position=(0, 0),
                )

            # ---- phase A: yT strips ----
            for k in range(NCHUNK):
                i = k // 2
                tau, j = i // 4, i % 4
                nc.tensor.matmul(
                    ytile(tau)[32 * j : 32 * j + 32, :],
                    lhsT=vt_sb[:, 32 * k : 32 * k + 32],
                    rhs=xchunk(k),
                    start=(k % 2 == 0),
                    stop=(k % 2 == 1),
                    tile_position=(0, 32 * j),
                )

            # ---- y -> SBUF (bf16), alternate DVE/ACT ----
            # tau=0 copy skips row 31 (holds the DMA'd const-1 row);
            # non-zero-start partition accesses are capped at 32 rows
            nc.vector.tensor_copy(ysb[0:31, 0:TPC], ytile(0)[0:31, :])
            for q in (32, 64, 96):
                nc.vector.tensor_copy(
                    ysb[q : q + 32, 0:TPC], ytile(0)[q : q + 32, :]
                )
            for t in range(1, NTAU):
                dst = ysb[:, TPC * t : TPC * (t + 1)]
                if t % 2 == 0:
                    nc.vector.tensor_copy(dst, ytile(t))
                else:
                    nc.scalar.copy(dst, ytile(t))

            # ---- z-step: zT = smat^T @ ysb ----
            # bank-major: a start=True clears the whole PSUM bank's
            # has_written, so groups sharing a bank must not interleave
            for b in range(NBANK):
                for tau in range(NTAU):
                    nc.tensor.matmul(
                        ztile(b),
                        lhsT=smat_sb[
                            :, 128 * (NBANK * tau + b) : 128 * (NBANK * tau + b + 1)
                        ],
                        rhs=ysb[:, TPC * tau : TPC * (tau + 1)],
                        start=(tau == 0),
                        stop=(tau == NTAU - 1),
                        tile_position=(0, 0),
                    )

            # ---- z -> SBUF ----
            for b in range(NBANK):
                dst = zsb[:, TPC * b : TPC * (b + 1)]
                if b % 2 == 0:
                    nc.vector.tensor_copy(dst, ztile(b))
                else:
                    nc.scalar.copy(dst, ztile(b))

            # ---- phase B: out[t, oq] per o-block; copies pace; flush out ----
            ps_out = ctx.enter_context(
                tc.tile_pool(name="ps_out", bufs=4, space="PSUM")
            )
            for half in range(2):
                osb_t = outsb.tile(
                    [128, OUT_DIM], bf16, name=f"osb{half}", tag=f"osb{half}"
                )
                row = slice(half * 128, (half + 1) * 128)
                for m in range(8):
                    po = ps_out.tile([128, 512], f32, name="po", tag="po")
                    for o in (2 * m, 2 * m + 1):
                        b, _ = _bank(o)
                        nc.tensor.matmul(
                            po[:, (o % 2) * BLOCK : (o % 2 + 1) * BLOCK],
                            lhsT=zsb[
                                :, TPC * b + 128 * half : TPC * b + 128 * (half + 1)
                            ],
                            rhs=usb[:, BLOCK * o : BLOCK * (o + 1)],
                            start=True,
                            stop=True,
                            tile_position=(0, 0),
                        )
                    dst = osb_t[:, 512 * m : 512 * (m + 1)]
                    if m % 2 == 0:
                        nc.vector.tensor_copy(dst, po[:])
                    else:
                        nc.scalar.copy(dst, po[:])
                    if m % 2 == 1:
                        c0 = 512 * (m - 1)
                        nc.sync.dma_start(
                            out=out_d[row, c0 : c0 + 1024],
                            in_=osb_t[:, c0 : c0 + 1024],
                        )

    nc.compile()
    return nc


def prep_inputs(x, S, U, Vt, bias):
    """Host-side layout prep (bf16). Returns per-core input maps."""
    import ml_dtypes

    bf = ml_dtypes.bfloat16
    x = np.asarray(x, dtype=np.float32)
    S = np.asarray(S, dtype=np.float32)
    U = np.asarray(U, dtype=np.float32)
    Vt = np.asarray(Vt, dtype=np.float32)
    bias = np.asarray(bias, dtype=np.float32)

    xt = np.ascontiguousarray(x.reshape(TOK, IN_DIM).T).astype(bf)  # (4096, 2048)

    # vt[p, 32k + c]: c<16 -> Vt[i, 128h+p, c] (k=2i+h); c==16 -> 1.0; else 0
    vt_host = np.zeros((128, NCHUNK, 32), np.float32)
    for k in range(NCHUNK):
        i, h = k // 2, k % 2
        vt_host[:, k, 0:RANK] = Vt[i, 128 * h : 128 * (h + 1), :]
        vt_host[:, k, 16] = 1.0
    vt_host = vt_host.reshape(128, NCHUNK * 32).astype(bf)

    # smat block (tau, b) at cols 128*(NBANK*tau + b):
    #   [32j + r, 32 o_loc + r] = S[o, 4 tau + j, r]
    #   [32j + 16, BROW] = 1.0 (rowsum accumulate); [31, BROW] = 1.0 on tau=0
    smat = np.zeros((128, NTAU * NBANK * 128), np.float32)
    for tau in range(NTAU):
        for b in range(NBANK):
            c0 = 128 * (NBANK * tau + b)
            for j in range(4):
                i = 4 * tau + j
                for o in range(4 * b, 4 * b + 4):
                    o_loc = o % 4
                    for r in range(RANK):
                        smat[32 * j + r, c0 + 32 * o_loc + r] = S[o, i, r]
                smat[32 * j + 16, c0 + BROW] = 1.0
            if tau == 0:
                smat[31, c0 + BROW] = 1.0
    smat = smat.astype(bf)

    # uc[32 o_loc + r, 256 b + q] = U[o, r, q] (pre-placed rows per bank)
    uc = np.zeros((128, NBANK * BLOCK), np.float32)
    for o in range(B_OUT):
        b, o_loc = _bank(o)
        uc[32 * o_loc : 32 * o_loc + RANK, BLOCK * b : BLOCK * (b + 1)] = U[o]
    uc = uc.astype(bf)

    br = bias.reshape(1, OUT_DIM).astype(bf)
    ones = np.ones((1, TPC), np.float32).astype(bf)

    in_maps = []
    for c in range(N_CORES):
        in_maps.append(
            {
                "xt": np.ascontiguousarray(
                    xt[:, c * TPC : (c + 1) * TPC]
                    .reshape(NCHUNK, 128, TPC)
                    .transpose(1, 0, 2)
                    .reshape(128, NCHUNK * TPC)
                ),
                "vt": vt_host,
                "smat": smat,
                "uc": uc,
                "br": br,
                "ones": ones,
            }
        )
    return in_maps


def kernel(x, S, U, Vt, bias):
    global LAST_RESULTS
    from concourse.bass_utils import run_bass_kernel_spmd

    if "nc" not in _CACHE:
        _CACHE["nc"] = build_program()
    nc = _CACHE["nc"]

    in_maps = prep_inputs(x, S, U, Vt, bias)
    res = run_bass_kernel_spmd(
        nc, in_maps, list(range(N_CORES)), trace=TRACE, tmpdir=TRACE_DIR
    )
    LAST_RESULTS = res
    out = np.concatenate(
        [np.asarray(res.results[c]["out"]).astype(np.float32) for c in range(N_CORES)],
        axis=0,
    )
    return out.reshape(2, TOK // 2, OUT_DIM)


# revision 15
# speedup vs baseline: 1.2687x; 1.1675x over previous
"""Trainium2 Bass kernel for nn_Blast: out = x @ (W0 + 1 bias^T) + bias
where W0 block (i_in, i_out) = Vt[i] @ diag(S[o,i]) @ U[o].

v10: y-factorization, token-half pipeline. Per core (256 tokens, 2
halves of 128):
  phase A_h: yT[(i,r), t] = blockdiag(Vt)^T @ xT_h  (32 mm, M=32 strips
             via tile_position col groups; col 16 = ones -> rowsum)
  z_h:       zT[(o,r), t] = smat^T @ ysb_h          (16 mm, bank-major)
  phase B_h: out_h = zsb_h^T-matmuls vs usb          (16 mm)
Halves pipeline: B(h0) copies + out0 DMA overlap x(h1) receipts.
z banks interleave with B banks (B bank b starts right after z bank b).

Measured constants this design works around: DMA receipts fire ~2.5-3us
after last byte (per dma_start); engine-queue order is program order
(in-order per engine); a matmul start=True clears the whole PSUM bank's
has_written (groups sharing a bank must not interleave); the framework
postamble costs ~8.3us after the last DMA receipt no matter what.

Weights ship as ONE packed DMA (vt|smat|uc -> fewer receipts), first on
the scalar ring. usb (block-diag U, bias row 16 via DMA) is built by 4
rearranged DVE copies. PSUM: 2 y banks + 2 z banks + 4 out banks = 8.
"""

import numpy as np

IN_DIM = 4096
OUT_DIM = 4096
BLOCK = 256
RANK = 16
B_IN = 16
B_OUT = 16
N_CORES = 8
TOK = 2048
TPC = TOK // N_CORES          # 256 tokens per core
HTOK = 128                    # tokens per half
NCHUNK = IN_DIM // 128        # 32 K-chunks
NTAU = 4                      # y tiles (4 i-blocks each)
NBANK = 4                     # z/usb banks: 4 o-blocks each, pitch 32
BROW = 16                     # bias/rowsum row in zsb/usb (per bank)
NXB = 2                       # x batches per half (16 chunks each)
NWARM = 26                    # K=128 N=256 warm matmuls (HAM un-throttle)
NFILL = 10                    # N=64 fillers after each x batch section

VT_C = NCHUNK * 32            # 1024 wpack cols for vt
SM_C = NTAU * NBANK * 128     # 2048 cols for smat
UC_C = NBANK * BLOCK          # 1024 cols for uc
WP_C = VT_C + SM_C + UC_C     # 4096

_CACHE = {}

# test.py toggles; harness never touches these
TRACE = False
TRACE_DIR = None
LAST_RESULTS = None


def _bank(o):
    return o // 4, o % 4


def build_program():
    import concourse.mybir as mybir
    from concourse import bacc
    from concourse.tile import TileContext

    bf16 = mybir.dt.bfloat16
    f32 = mybir.dt.float32

    nc = bacc.Bacc(trn_type="TRN2")
    # xt[p, half*4096 + k*128 + t] = x^T[128k+p, 128*half + t]
    xt_d = nc.dram_tensor("xt", (128, NCHUNK * TPC), bf16, kind="ExternalInput")
    wp_d = nc.dram_tensor("wp", (128, WP_C), bf16, kind="ExternalInput")
    br_d = nc.dram_tensor("br", (1, OUT_DIM), bf16, kind="ExternalInput")
    ones_d = nc.dram_tensor("ones", (1, TPC), bf16, kind="ExternalInput")
    out_d = nc.dram_tensor("out", (TPC, OUT_DIM), bf16, kind="ExternalOutput")

    with TileContext(nc) as tc:
        from contextlib import ExitStack

        with ExitStack() as ctx:
            consts = ctx.enter_context(tc.tile_pool(name="consts", bufs=1))
            xpool = ctx.enter_context(tc.tile_pool(name="xpool", bufs=1))
            outsb = ctx.enter_context(tc.tile_pool(name="outsb", bufs=1))
            ps_y = ctx.enter_context(tc.tile_pool(name="ps_y", bufs=1, space="PSUM"))
            ps_z = ctx.enter_context(tc.tile_pool(name="ps_z", bufs=1, space="PSUM"))

            # ---- SBUF tiles ----
            wsrc = consts.tile([128, TPC], bf16, name="wsrc", tag="wsrc")
            nc.vector.memset(wsrc[:], 0.0)

            wp_sb = consts.tile([128, WP_C], bf16, name="wp_sb", tag="wp_sb")
            vt_sb = wp_sb[:, 0:VT_C]
            smat_sb = wp_sb[:, VT_C : VT_C + SM_C]
            uc_sb = wp_sb[:, VT_C + SM_C : WP_C]
            usb = consts.tile([128, OUT_DIM], bf16, name="usb", tag="usb")
            ysb = consts.tile([128, NTAU * TPC], bf16, name="ysb", tag="ysb")
            zsb = consts.tile([128, NBANK * TPC], bf16, name="zsb", tag="zsb")

            nc.vector.memset(usb[:], 0.0)

            # ---- input DMAs ----
            # scalar ring: packed weights first (one receipt), then ones
            nc.scalar.dma_start(out=wp_sb[:], in_=wp_d[:])
            nc.scalar.dma_start(out=ysb[31:32, 0:TPC], in_=ones_d[:])

            # sync ring: x batches, half-major (16 chunks = 512KB each)
            xbat = {}
            for h in range(2):
                for bi in range(NXB):
                    xb = xpool.tile(
                        [128, 16 * HTOK], bf16, name=f"xb{h}_{bi}", tag=f"xb{h}_{bi}"
                    )
                    c0 = h * NCHUNK * HTOK + bi * 16 * HTOK
                    nc.sync.dma_start(out=xb[:], in_=xt_d[:, c0 : c0 + 16 * HTOK])
                    xbat[(h, bi)] = xb

            def xchunk(h, k):
                xb = xbat[(h, k // 16)]
                return xb[:, (k % 16) * HTOK : (k % 16 + 1) * HTOK]

            # ---- usb build on DVE: 4 rearranged copies (one per o_loc)
            # dst cols {1024b + 256 o_loc + q}, src cols {256b + q} ----
            usb_v = usb[:].rearrange("p (b oq q) -> p b oq q", b=NBANK, q=BLOCK)
            uc_v = uc_sb.rearrange("p (b q) -> p b q", b=NBANK)
            for o_loc in range(4):
                r0 = 32 * o_loc
                nc.vector.tensor_copy(
                    usb_v[r0 : r0 + 32, :, o_loc, :],
                    uc_v[r0 : r0 + 32, :, :],
                )
            # bias row AFTER the o_loc=0 copy (covers row 16)
            nc.scalar.dma_start(out=usb[BROW : BROW + 1, :], in_=br_d[:])

            # PSUM: 2 y banks + 2 z banks + 4 out banks = 8 exactly.
            ypair = [
                ps_y.tile([128, 2 * TPC], f32, name=f"yp{t}", tag=f"yp{t}")
                for t in range(2)
            ]
            zpair = [
                ps_z.tile([128, 2 * TPC], f32, name=f"zp{t}", tag=f"zp{t}")
                for t in range(2)
            ]

            def ytile(tau, h):
                # [128, 128] slot per (tau, half)
                return ypair[tau // 2][
                    :, TPC * (tau % 2) + HTOK * h : TPC * (tau % 2) + HTOK * (h + 1)
                ]

            def ztile(b, h):
                return zpair[b // 2][
                    :, TPC * (b % 2) + HTOK * h : TPC * (b % 2) + HTOK * (h + 1)
                ]

            # ---- PE warmup (HAM un-throttle) ----
            warm = zpair[1][:, 0:TPC]
            for _ in range(NWARM):
                nc.tensor.matmul(
                    warm,
                    lhsT=wsrc[:, 0:128],
                    rhs=wsrc[:],
                    start=True,
                    stop=True,
                    tile_position=(0, 0),
                )

            def filler(n):
                for _ in range(n):
                    nc.tensor.matmul(
                        warm[0:128, 0:64],
                        lhsT=wsrc[:, 0:128],
                        rhs=wsrc[:, 0:64],
                        start=True,
                        stop=True,
                        tile_position=(0, 0),
                    )

            ps_out = ctx.enter_context(
                tc.tile_pool(name="ps_out", bufs=4, space="PSUM")
            )
            osb = [
                outsb.tile([128, OUT_DIM], bf16, name=f"osb{h}", tag=f"osb{h}")
                for h in range(2)
            ]

            def phase_a(h):
                for k in range(NCHUNK):
                    i = k // 2
                    tau, j = i // 4, i % 4
                    nc.tensor.matmul(
                        ytile(tau, h)[32 * j : 32 * j + 32, :],
                        lhsT=vt_sb[:, 32 * k : 32 * k + 32],
                        rhs=xchunk(h, k),
                        start=(k % 2 == 0),
                        stop=(k % 2 == 1),
                        tile_position=(0, 32 * j),
                    )
                    if k == 15:
                        filler(NFILL)
                filler(NFILL)

            def y_copies(h):
                # tau=0 skips row 31 (const-1 row); alternate DVE/ACT
                nc.vector.tensor_copy(
                    ysb[0:31, HTOK * h : HTOK * (h + 1)], ytile(0, h)[0:31, :]
                )
                for q in (32, 64, 96):
                    nc.vector.tensor_copy(
                        ysb[q : q + 32, HTOK * h : HTOK * (h + 1)],
                        ytile(0, h)[q : q + 32, :],
                    )
                for t in range(1, NTAU):
                    dst = ysb[:, TPC * t + HTOK * h : TPC * t + HTOK * (h + 1)]
                    if t % 2 == 0:
                        nc.vector.tensor_copy(dst, ytile(t, h))
                    else:
                        nc.scalar.copy(dst, ytile(t, h))

            def z_bank(h, b):
                for tau in range(NTAU):
                    nc.tensor.matmul(
                        ztile(b, h),
                        lhsT=smat_sb[
                            :, 128 * (NBANK * tau + b) : 128 * (NBANK * tau + b + 1)
                        ],
                        rhs=ysb[:, TPC * tau + HTOK * h : TPC * tau + HTOK * (h + 1)],
                        start=(tau == 0),
                        stop=(tau == NTAU - 1),
                        tile_position=(0, 0),
                    )
                dst = zsb[:, TPC * b + HTOK * h : TPC * b + HTOK * (h + 1)]
                if b % 2 == 0:
                    nc.vector.tensor_copy(dst, ztile(b, h))
                else:
                    nc.scalar.copy(dst, ztile(b, h))

            def b_bank(h, b):
                # 4 o-blocks; two [128,512] psum tiles; one [128,1024] piece
                for m in (2 * b, 2 * b + 1):
                    po = ps_out.tile([128, 512], f32, name="po", tag="po")
                    for o in (2 * m, 2 * m + 1):
                        nc.tensor.matmul(
                            po[:, (o % 2) * BLOCK : (o % 2 + 1) * BLOCK],
                            lhsT=zsb[
                                :, TPC * b + HTOK * h : TPC * b + HTOK * (h + 1)
                            ],
                            rhs=usb[:, BLOCK * o : BLOCK * (o + 1)],
                            start=True,
                            stop=True,
                            tile_position=(0, 0),
                        )
                    dst = osb[h][:, 512 * m : 512 * (m + 1)]
                    if m % 2 == 0:
                        nc.vector.tensor_copy(dst, po[:])
                    else:
                        nc.scalar.copy(dst, po[:])
                c0 = 1024 * b
                nc.sync.dma_start(
                    out=out_d[HTOK * h : HTOK * (h + 1), c0 : c0 + 1024],
                    in_=osb[h][:, c0 : c0 + 1024],
                )

            # ---- pipeline: A0, (z0|B0 interleaved), A1, (z1|B1) ----
            phase_a(0)
            y_copies(0)
            z_bank(0, 0)
            z_bank(0, 1)
            b_bank(0, 0)
            z_bank(0, 2)
            b_bank(0, 1)
            z_bank(0, 3)
            b_bank(0, 2)
            b_bank(0, 3)
            phase_a(1)
            y_copies(1)
            z_bank(1, 0)
            z_bank(1, 1)
            b_bank(1, 0)
            z_bank(1, 2)
            b_bank(1, 1)
            z_bank(1, 3)
            b_bank(1, 2)
            b_bank(1, 3)

    nc.compile()
    return nc


def prep_inputs(x, S, U, Vt, bias):
    """Host-side layout prep (bf16). Returns per-core input maps."""
    import ml_dtypes

    bf = ml_dtypes.bfloat16
    x = np.asarray(x, dtype=np.float32)
    S = np.asarray(S, dtype=np.float32)
    U = np.asarray(U, dtype=np.float32)
    Vt = np.asarray(Vt, dtype=np.float32)
    bias = np.asarray(bias, dtype=np.float32)

    xt = np.ascontiguousarray(x.reshape(TOK, IN_DIM).T).astype(bf)  # (4096, 2048)

    # vt[p, 32k + c]: c<16 -> Vt[i, 128h+p, c] (k=2i+h); c==16 -> 1.0; else 0
    vt_host = np.zeros((128, NCHUNK, 32), np.float32)
    for k in range(NCHUNK):
        i, h = k // 2, k % 2
        vt_host[:, k, 0:RANK] = Vt[i, 128 * h : 128 * (h + 1), :]
        vt_host[:, k, 16] = 1.0
    vt_host = vt_host.reshape(128, VT_C)

    # smat block (tau, b): [32j + r, 32 o_loc + r] = S[o, 4 tau + j, r];
    # [32j + 16, BROW] = 1.0; [31, BROW] = 1.0 on tau=0
    smat = np.zeros((128, SM_C), np.float32)
    for tau in range(NTAU):
        for b in range(NBANK):
            c0 = 128 * (NBANK * tau + b)
            for j in range(4):
                i = 4 * tau + j
                for o in range(4 * b, 4 * b + 4):
                    o_loc = o % 4
                    for r in range(RANK):
                        smat[32 * j + r, c0 + 32 * o_loc + r] = S[o, i, r]
                smat[32 * j + 16, c0 + BROW] = 1.0
            if tau == 0:
                smat[31, c0 + BROW] = 1.0

    # uc[32 o_loc + r, 256 b + q] = U[o, r, q]
    uc = np.zeros((128, UC_C), np.float32)
    for o in range(B_OUT):
        b, o_loc = _bank(o)
        uc[32 * o_loc : 32 * o_loc + RANK, BLOCK * b : BLOCK * (b + 1)] = U[o]

    wp = np.concatenate([vt_host, smat, uc], axis=1).astype(bf)
    br = bias.reshape(1, OUT_DIM).astype(bf)
    ones = np.ones((1, TPC), np.float32).astype(bf)

    in_maps = []
    for c in range(N_CORES):
        # xt[p, h*4096 + k*128 + t] = xT[128k+p, 256c + 128h + t]
        xc = (
            xt[:, c * TPC : (c + 1) * TPC]
            .reshape(NCHUNK, 128, 2, HTOK)
            .transpose(1, 2, 0, 3)
            .reshape(128, NCHUNK * TPC)
        )
        in_maps.append(
            {
                "xt": np.ascontiguousarray(xc),
                "wp": wp,
                "br": br,
                "ones": ones,
            }
        )
    return in_maps


def kernel(x, S, U, Vt, bias):
    global LAST_RESULTS
    from concourse.bass_utils import run_bass_kernel_spmd

    if "nc" not in _CACHE:
        _CACHE["nc"] = build_program()
    nc = _CACHE["nc"]

    in_maps = prep_inputs(x, S, U, Vt, bias)
    res = run_bass_kernel_spmd(
        nc, in_maps, list(range(N_CORES)), trace=TRACE, tmpdir=TRACE_DIR
    )
    LAST_RESULTS = res
    out = np.concatenate(
        [np.asarray(res.results[c]["out"]).astype(np.float32) for c in range(N_CORES)],
        axis=0,
    )
    return out.reshape(2, TOK // 2, OUT_DIM)


# revision 19
# speedup vs baseline: 1.2733x; 1.0036x over previous
"""Trainium2 Bass kernel for nn_Blast: out = x @ (W0 + 1 bias^T) + bias
where W0 block (i_in, i_out) = Vt[i] @ diag(S[o,i]) @ U[o].

v11: y-factorization, token-half pipeline, receipt-aware DMA layout.

Per core (256 tokens, 2 halves of 128):
  phase A_h: yT[(i,r), t] = blockdiag(Vt)^T @ xT_h  (32 mm, M=32 strips
             via tile_position col groups; col 16 = ones -> rowsum)
  z_h:       zT[(o,r), t] = smat^T @ ysb_h          (16 mm, bank-major)
  phase B_h: out_h = zsb_h vs usb matmuls           (16 mm)
B(h0) copies + out0 DMA overlap x(h1); z bank b feeds B bank b directly.

Empirical HW laws this is built around:
 - DMA receipts fire ~0.8us (64-256KB) to ~3.5us (1MB) after last byte;
   so x ships as 4x4-chunk batches per half and weights as several small
   DMAs, critical ones first (vt, ones, uc, smat halves, bias).
 - Engine queues are strictly in-order: copy-engine (DVE/ACT) programs
   interleave B(h0) copies with y(h1) copies to avoid head-of-line
   blocking; usb build copies sit before any y copy (uc arrives early).
 - matmul start=True clears the whole PSUM bank's has_written: groups
   sharing a bank never interleave.
 - Engine partition access: start must be 32-aligned (or 0); non-zero
   start caps span at 32 rows.
 - Framework postamble costs ~8.3us after the last DMA receipt, always.
 - gpsimd ~1us/op (only gets one big memset); DVE memset runs at 1x so
   the usb memset is split DVE/GPS.
bias trick: uc row 16 carries bias for o_loc=0 blocks (so the block
copies place it); a strided DMA fills row 16 of o_loc=1..3 col-blocks
(disjoint from all copies -> no WAW, issues early).
"""

import numpy as np

IN_DIM = 4096
OUT_DIM = 4096
BLOCK = 256
RANK = 16
B_IN = 16
B_OUT = 16
N_CORES = 8
TOK = 2048
TPC = TOK // N_CORES          # 256 tokens per core
HTOK = 128                    # tokens per half
NCHUNK = IN_DIM // 128        # 32 K-chunks
NTAU = 4                      # y tiles (4 i-blocks each)
NBANK = 4                     # z/usb banks: 4 o-blocks each, pitch 32
BROW = 16                     # bias/rowsum row in zsb/usb (per bank)
XB_CH = 8                     # chunks per x batch (4 batches per half)
NWARM = 26                    # K=128 N=256 warm matmuls (HAM un-throttle)
NFILL = 6                     # N=64 fillers after each x batch section

VT_C = NCHUNK * 32            # 1024
SM_C = NTAU * NBANK * 128     # 2048 (bank-major blocks: idx 4b+tau)
UC_C = NBANK * BLOCK          # 1024

_CACHE = {}

# test.py toggles; harness never touches these
TRACE = False
TRACE_DIR = None
LAST_RESULTS = None


def _bank(o):
    return o // 4, o % 4


def build_program():
    import concourse.mybir as mybir
    from concourse import bacc
    from concourse.tile import TileContext

    bf16 = mybir.dt.bfloat16
    f32 = mybir.dt.float32

    nc = bacc.Bacc(trn_type="TRN2")
    # xt[p, half*4096 + k*128 + t] = x^T[128k+p, 128*half + t]
    xt_d = nc.dram_tensor("xt", (128, NCHUNK * TPC), bf16, kind="ExternalInput")
    vt_d = nc.dram_tensor("vt", (128, VT_C), bf16, kind="ExternalInput")
    sm_d = nc.dram_tensor("sm", (128, SM_C), bf16, kind="ExternalInput")
    uc_d = nc.dram_tensor("uc", (128, UC_C), bf16, kind="ExternalInput")
    br_d = nc.dram_tensor("br", (1, 12 * BLOCK), bf16, kind="ExternalInput")
    ones_d = nc.dram_tensor("ones", (1, TPC), bf16, kind="ExternalInput")
    out_d = nc.dram_tensor("out", (TPC, OUT_DIM), bf16, kind="ExternalOutput")

    with TileContext(nc) as tc:
        from contextlib import ExitStack

        with ExitStack() as ctx:
            consts = ctx.enter_context(tc.tile_pool(name="consts", bufs=1))
            xpool = ctx.enter_context(tc.tile_pool(name="xpool", bufs=1))
            outsb = ctx.enter_context(tc.tile_pool(name="outsb", bufs=1))
            ps_y = ctx.enter_context(tc.tile_pool(name="ps_y", bufs=1, space="PSUM"))
            ps_z = ctx.enter_context(tc.tile_pool(name="ps_z", bufs=1, space="PSUM"))

            # ---- SBUF tiles ----
            wsrc = consts.tile([128, TPC], bf16, name="wsrc", tag="wsrc")
            nc.vector.memset(wsrc[:], 0.0)

            vt_sb = consts.tile([128, VT_C], bf16, name="vt_sb", tag="vt_sb")
            smat_sb = consts.tile([128, SM_C], bf16, name="smat_sb", tag="smat_sb")
            uc_sb = consts.tile([128, UC_C], bf16, name="uc_sb", tag="uc_sb")
            usb = consts.tile([128, OUT_DIM], bf16, name="usb", tag="usb")
            ysb = consts.tile([128, NTAU * TPC], bf16, name="ysb", tag="ysb")
            zsb = consts.tile([128, NBANK * TPC], bf16, name="zsb", tag="zsb")

            # usb zeroing split across DVE and GPS (both run at ~1x)
            nc.vector.memset(usb[:, 0:2048], 0.0)
            nc.gpsimd.memset(usb[:, 2048:4096], 0.0)

            # ---- input DMAs (scalar ring, receipt-critical first) ----
            nc.scalar.dma_start(out=vt_sb[:], in_=vt_d[:])
            nc.scalar.dma_start(out=ysb[31:32, 0:TPC], in_=ones_d[:])
            nc.scalar.dma_start(out=uc_sb[:], in_=uc_d[:])
            nc.scalar.dma_start(out=smat_sb[:, 0:1024], in_=sm_d[:, 0:1024])
            nc.scalar.dma_start(out=smat_sb[:, 1024:2048], in_=sm_d[:, 1024:2048])
            # bias for o_loc=1..3 col-blocks: row 16, disjoint from the
            # usb block copies (o_loc=0 bias arrives inside uc row 16)
            usb_v = usb[:].rearrange("p (b oq q) -> p b oq q", b=NBANK, q=BLOCK)
            nc.scalar.dma_start(
                out=usb_v[BROW : BROW + 1, :, 1:4, :],
                in_=br_d[:].rearrange("p (b oq q) -> p b oq q", b=NBANK, q=BLOCK),
            )

            # sync ring: x batches, half-major (4 chunks = 128KB each)
            xbat = {}
            for h in range(2):
                for bi in range(NCHUNK // XB_CH):
                    xb = xpool.tile(
                        [128, XB_CH * HTOK], bf16,
                        name=f"xb{h}_{bi}", tag=f"xb{h}_{bi}",
                    )
                    c0 = h * NCHUNK * HTOK + bi * XB_CH * HTOK
                    nc.sync.dma_start(
                        out=xb[:], in_=xt_d[:, c0 : c0 + XB_CH * HTOK]
                    )
                    xbat[(h, bi)] = xb

            def xchunk(h, k):
                xb = xbat[(h, k // XB_CH)]
                return xb[:, (k % XB_CH) * HTOK : (k % XB_CH + 1) * HTOK]

            # ---- usb build: 4 rearranged DVE copies (one per o_loc) ----
            uc_v = uc_sb[:].rearrange("p (b q) -> p b q", b=NBANK)
            for o_loc in range(4):
                r0 = 32 * o_loc
                nc.vector.tensor_copy(
                    usb_v[r0 : r0 + 32, :, o_loc, :],
                    uc_v[r0 : r0 + 32, :, :],
                )

            # PSUM: 2 y banks + 2 z banks + 4 out banks = 8 exactly.
            ypair = [
                ps_y.tile([128, 2 * TPC], f32, name=f"yp{t}", tag=f"yp{t}")
                for t in range(2)
            ]
            zpair = [
                ps_z.tile([128, 2 * TPC], f32, name=f"zp{t}", tag=f"zp{t}")
                for t in range(2)
            ]

            def ytile(tau, h):
                return ypair[tau // 2][
                    :, TPC * (tau % 2) + HTOK * h : TPC * (tau % 2) + HTOK * (h + 1)
                ]

            def ztile(b, h):
                return zpair[b // 2][
                    :, TPC * (b % 2) + HTOK * h : TPC * (b % 2) + HTOK * (h + 1)
                ]

            # ---- PE warmup ----
            warm = zpair[1][:, 0:TPC]
            for _ in range(NWARM):
                nc.tensor.matmul(
                    warm, lhsT=wsrc[:, 0:128], rhs=wsrc[:],
                    start=True, stop=True, tile_position=(0, 0),
                )

            def filler(n):
                for _ in range(n):
                    nc.tensor.matmul(
                        warm[0:128, 0:64], lhsT=wsrc[:, 0:128],
                        rhs=wsrc[:, 0:64],
                        start=True, stop=True, tile_position=(0, 0),
                    )

            ps_out = ctx.enter_context(
                tc.tile_pool(name="ps_out", bufs=4, space="PSUM")
            )
            osb = [
                outsb.tile([128, OUT_DIM], bf16, name=f"osb{h}", tag=f"osb{h}")
                for h in range(2)
            ]

            # ---------------- PE program ----------------
            def phase_a(h):
                for k in range(NCHUNK):
                    i = k // 2
                    tau, j = i // 4, i % 4
                    nc.tensor.matmul(
                        ytile(tau, h)[32 * j : 32 * j + 32, :],
                        lhsT=vt_sb[:, 32 * k : 32 * k + 32],
                        rhs=xchunk(h, k),
                        start=(k % 2 == 0),
                        stop=(k % 2 == 1),
                        tile_position=(0, 32 * j),
                    )
                    if k % XB_CH == XB_CH - 1 and k < NCHUNK - 1:
                        filler(NFILL)

            def z_bank(h, b):
                for tau in range(NTAU):
                    nc.tensor.matmul(
                        ztile(b, h),
                        lhsT=smat_sb[
                            :, 128 * (NTAU * b + tau) : 128 * (NTAU * b + tau + 1)
                        ],
                        rhs=ysb[:, TPC * tau + HTOK * h : TPC * tau + HTOK * (h + 1)],
                        start=(tau == 0),
                        stop=(tau == NTAU - 1),
                        tile_position=(0, 0),
                    )

            def b_bank(h, b):
                for m in (2 * b, 2 * b + 1):
                    po = ps_out.tile([128, 512], f32, name="po", tag="po")
                    for o in (2 * m, 2 * m + 1):
                        nc.tensor.matmul(
                            po[:, (o % 2) * BLOCK : (o % 2 + 1) * BLOCK],
                            lhsT=zsb[
                                :, TPC * b + HTOK * h : TPC * b + HTOK * (h + 1)
                            ],
                            rhs=usb[:, BLOCK * o : BLOCK * (o + 1)],
                            start=True, stop=True, tile_position=(0, 0),
                        )
                    yield po, m

            # ---------------- copy helpers (engine-explicit) ----------------
            def y_copy_dve(h, t):
                # tau 0 skips row 31 (const-1 row, DMA'd)
                if t == 0:
                    nc.vector.tensor_copy(
                        ysb[0:31, HTOK * h : HTOK * (h + 1)], ytile(0, h)[0:31, :]
                    )
                    for q in (32, 64, 96):
                        nc.vector.tensor_copy(
                            ysb[q : q + 32, HTOK * h : HTOK * (h + 1)],
                            ytile(0, h)[q : q + 32, :],
                        )
                else:
                    nc.vector.tensor_copy(
                        ysb[:, TPC * t + HTOK * h : TPC * t + HTOK * (h + 1)],
                        ytile(t, h),
                    )

            def y_copy_act(h, t):
                nc.scalar.copy(
                    ysb[:, TPC * t + HTOK * h : TPC * t + HTOK * (h + 1)],
                    ytile(t, h),
                )

            def z_copy(h, b):
                dst = zsb[:, TPC * b + HTOK * h : TPC * b + HTOK * (h + 1)]
                if b % 2 == 0:
                    nc.vector.tensor_copy(dst, ztile(b, h))
                else:
                    nc.scalar.copy(dst, ztile(b, h))

            def o_copy(h, po, m):
                dst = osb[h][:, 512 * m : 512 * (m + 1)]
                if m % 2 == 0:
                    nc.vector.tensor_copy(dst, po[:])
                else:
                    nc.scalar.copy(dst, po[:])

            def o_piece(h, b):
                c0 = 1024 * b
                nc.sync.dma_start(
                    out=out_d[HTOK * h : HTOK * (h + 1), c0 : c0 + 1024],
                    in_=osb[h][:, c0 : c0 + 1024],
                )

            def half(h):
                phase_a(h)
                y_copy_dve(h, 0)
                y_copy_act(h, 1)
                y_copy_dve(h, 2)
                y_copy_act(h, 3)
                for b in range(NBANK):
                    z_bank(h, b)
                    z_copy(h, b)
                    for po, m in b_bank(h, b):
                        o_copy(h, po, m)
                    o_piece(h, b)

            half(0)
            half(1)

    nc.compile()
    return nc


def prep_inputs(x, S, U, Vt, bias):
    """Host-side layout prep (bf16). Returns per-core input maps."""
    import ml_dtypes

    bf = ml_dtypes.bfloat16
    x = np.asarray(x, dtype=np.float32)
    S = np.asarray(S, dtype=np.float32)
    U = np.asarray(U, dtype=np.float32)
    Vt = np.asarray(Vt, dtype=np.float32)
    bias = np.asarray(bias, dtype=np.float32)

    xt = np.ascontiguousarray(x.reshape(TOK, IN_DIM).T).astype(bf)  # (4096, 2048)

    # vt[p, 32k + c]: c<16 -> Vt[i, 128h+p, c] (k=2i+h); c==16 -> 1.0; else 0
    vt_host = np.zeros((128, NCHUNK, 32), np.float32)
    for k in range(NCHUNK):
        i, h = k // 2, k % 2
        vt_host[:, k, 0:RANK] = Vt[i, 128 * h : 128 * (h + 1), :]
        vt_host[:, k, 16] = 1.0
    vt_host = vt_host.reshape(128, VT_C).astype(bf)

    # smat bank-major: block (b, tau) at cols 128*(NTAU*b + tau)
    smat = np.zeros((128, SM_C), np.float32)
    for b in range(NBANK):
        for tau in range(NTAU):
            c0 = 128 * (NTAU * b + tau)
            for j in range(4):
                i = 4 * tau + j
                for o in range(4 * b, 4 * b + 4):
                    o_loc = o % 4
                    for r in range(RANK):
                        smat[32 * j + r, c0 + 32 * o_loc + r] = S[o, i, r]
                smat[32 * j + 16, c0 + BROW] = 1.0
            if tau == 0:
                smat[31, c0 + BROW] = 1.0
    smat = smat.astype(bf)

    # uc[32 o_loc + r, 256 b + q] = U[o, r, q]; row 16 of block b carries
    # bias for o = 4b (the o_loc=0 usb copy places it)
    uc = np.zeros((128, UC_C), np.float32)
    for o in range(B_OUT):
        b, o_loc = _bank(o)
        uc[32 * o_loc : 32 * o_loc + RANK, BLOCK * b : BLOCK * (b + 1)] = U[o]
    for b in range(NBANK):
        uc[BROW, BLOCK * b : BLOCK * (b + 1)] = bias[BLOCK * 4 * b : BLOCK * (4 * b + 1)]
    uc = uc.astype(bf)

    # br2[b, o_loc-1, q] = bias[256*(4b + o_loc) + q] for o_loc 1..3
    br2 = np.zeros((NBANK, 3, BLOCK), np.float32)
    for b in range(NBANK):
        for ol in (1, 2, 3):
            br2[b, ol - 1] = bias[BLOCK * (4 * b + ol) : BLOCK * (4 * b + ol + 1)]
    br2 = br2.reshape(1, 12 * BLOCK).astype(bf)

    ones = np.ones((1, TPC), np.float32).astype(bf)

    in_maps = []
    for c in range(N_CORES):
        xc = (
            xt[:, c * TPC : (c + 1) * TPC]
            .reshape(NCHUNK, 128, 2, HTOK)
            .transpose(1, 2, 0, 3)
            .reshape(128, NCHUNK * TPC)
        )
        in_maps.append(
            {
                "xt": np.ascontiguousarray(xc),
                "vt": vt_host,
                "sm": smat,
                "uc": uc,
                "br": br2,
                "ones": ones,
            }
        )
    return in_maps


def kernel(x, S, U, Vt, bias):
    global LAST_RESULTS
    from concourse.bass_utils import run_bass_kernel_spmd

    if "nc" not in _CACHE:
        _CACHE["nc"] = build_program()
    nc = _CACHE["nc"]

    in_maps = prep_inputs(x, S, U, Vt, bias)
    res = run_bass_kernel_spmd(
        nc, in_maps, list(range(N_CORES)), trace=TRACE, tmpdir=TRACE_DIR
    )
    LAST_RESULTS = res
    out = np.concatenate(
        [np.asarray(res.results[c]["out"]).astype(np.float32) for c in range(N_CORES)],
        axis=0,
    )
    return out.reshape(2, TOK // 2, OUT_DIM)
